# revision 31
# baseline (speedup 1.0000x reference)
"""Trainium2 Bass kernel for nn_Message_Passer (gnn_message_passing).

Reference computation:
    A = relu(edge_ij @ W + b)            # [B, E, 1024]
    messages = einsum("beij,bej->bei", A.reshape(B,E,32,32), node_j)

Row-tiled design (8 NeuronCores, data-parallel over the B*E edge dim):
  - matmul1 uses PE row tiling: contraction is the 64 edge features (the bias
    is dropped — b is zeros for this problem; a numpy fallback handles b != 0)
    and the 128x128 PE array runs two concurrent 64-row tiles: rows 0-63
    process the X half of the core's edges (0..E/2), rows 64-127 the Y half,
    with W duplicated across both row halves.  Each PSUM pair is [128, 2*ET]
    = A-bank g for (X-tile | Y-tile); a super-tile is 8 pairs covering all 8
    W-blocks for 1024 edges.
  - Fused relu+multiply P = max(A,0) * node_rep, split across engines by a
    per-pair schedule: DVE scalar_tensor_tensor straight from PSUM ('s'),
    ACT relu + DVE bf16 multiply ('a'), ACT relu + GPSIMD multiply ('p') —
    balanced so DVE/ACT/Pool all stay busy.
  - j-reduction on the PE: 0/1 selector matmuls, col-tiled 4-wide and emitted
    as strip-interleaved runs one super-tile behind the fills so the in-order
    PE queue never waits on the elementwise engines.
  - Per half-tile: one [128, ET] PSUM->SBUF bf16 copy + one full-bank DMA
    store (partial-partition strip DMAs silently corrupt — DMA-queue race).
"""

import threading

import numpy as np
import ml_dtypes

import concourse.bass as bass
import concourse.mybir as mybir
import concourse.tile as tile
from concourse import bacc
from concourse.bass import ts, ds
from concourse.bass_utils import run_bass_kernel_spmd

N_CORES = 8
B, E_FULL, ND, ED = 16, 4096, 32, 64
EDGES = B * E_FULL            # 65536
E_CORE = EDGES // N_CORES     # 8192
EH = E_CORE // 2              # 4096 edges per half (X / Y)
ET = 512                      # edges per on-chip tile (per half)
NS = EH // ET                 # 8 super-tiles
GT = 4                        # super-tiles per input-load group
GRP = GT * ET                 # 2048 cols per half per load group
KDIM = ED                     # 64 (features; contraction dim)
NK = ND * ND                  # 1024 A-columns
F32 = mybir.dt.float32
F32R = mybir.dt.float32r
BF16 = mybir.dt.bfloat16

OUT_NAME = "msg_out"

# Per bank-pair engine schedule (cycled): 's' = DVE scalar_tensor_tensor,
# 'a' = ACT relu + DVE mult, 'p' = ACT relu + GPSIMD mult.  Tuned on HW.
# Period 24 = three 8-pair super-tiles, same fractions as 'saasap'
# (1/3 s, 1/2 a, 1/6 p) but with the slow GPSIMD pairs pinned early in
# each super-tile so the selector runs never wait on a Pool op.
SCHED = list("spaasaas" "sppasaaa" "spaasaas")
# Engine for each msg PSUM->SBUF copy (cycled): 'A' = ACT, 'V' = DVE.
COPY_SCHED = list("AAV")
# After which pair of super s+1 the X-sel run (resp. Y) of super s is emitted
SELX_AFTER = 0
SELY_AFTER = 1
# fp32r edges/weights measured faster than bf16 (bf16 FWL weight loads
# contend for XBUSes with the col-tiled selector matmuls)
EDGE_BF16 = False
EDT = BF16 if EDGE_BF16 else F32R


def _build_nc(repeat: int = 1):
    nc = bacc.Bacc("TRN2", target_bir_lowering=False, debug=False,
                   num_devices=N_CORES)
    # edge features split into 32-row tensors per half (proven-safe DMA
    # shapes: full-row-range source, col offsets <= 24KB)
    edge_ds = [nc.dram_tensor(f"edgeT{h}{r}", [32, EH], EDT,
                              kind="ExternalInput")
               for h in range(2) for r in range(2)]  # X0,X1,Y0,Y1
    nodeX_d = nc.dram_tensor("nodeX", [ND, EH], BF16, kind="ExternalInput")
    nodeY_d = nc.dram_tensor("nodeY", [ND, EH], BF16, kind="ExternalInput")
    w_d = nc.dram_tensor("w_pack", [128, NK], EDT, kind="ExternalInput")
    sel_d = nc.dram_tensor("sel", [128, 8 * ND], BF16, kind="ExternalInput")
    out_d = nc.dram_tensor(OUT_NAME, [128, E_CORE], BF16, kind="ExternalOutput")

    with tile.TileContext(nc) as tc:
        with (
            tc.tile_pool(name="const", bufs=1) as constp,
            tc.tile_pool(name="edge", bufs=3) as edgep,
            tc.tile_pool(name="node", bufs=3) as nodep,
            tc.tile_pool(name="ar", bufs=6) as arp,
            tc.tile_pool(name="pp", bufs=12) as ppp,
            tc.tile_pool(name="mo", bufs=6) as mop,
            tc.tile_pool(name="apsum", bufs=3, space="PSUM") as apsum,
            tc.tile_pool(name="mpsum", bufs=2, space="PSUM") as mpsum,
        ):
            w_sb = constp.tile([128, NK], EDT, name="w_sb")
            nc.sync.dma_start(out=w_sb[:], in_=w_d[:])
            sel_sb = constp.tile([128, 8 * ND], BF16, name="sel_sb")
            sel_loaded = False

            # pending super-tile sel work: [mgX, mgY, s, pps, stage]
            pend = []

            def emit_sels(entry, half):
                mgs, s_, pps = entry[0:2], entry[2], entry[3]
                mg_ = mgs[half]
                # even banks (start) then odd banks (stop): runs of 4
                # distinct col-tile positions overlap on the PE array
                for par in range(2):
                    for c_ in range(4):
                        g_ = 2 * c_ + par
                        nc.tensor.matmul(mg_[32 * c_:32 * (c_ + 1), :],
                                         sel_sb[:, ts(g_, ND)],
                                         pps[g_][:, ts(half, ET)],
                                         start=(par == 0), stop=(par == 1),
                                         skip_group_check=True,
                                         tile_position=(0, 32 * c_))
                # copy + store this half's messages
                mo = mop.tile([128, ET], BF16, name="mo")
                if COPY_SCHED[(2 * s_ + half) % len(COPY_SCHED)] == 'V':
                    nc.vector.tensor_copy(mo[:], mg_[:])
                else:
                    nc.scalar.copy(mo[:], mg_[:])
                nc.sync.dma_start(
                    out=out_d[:, ts(half * NS + s_, ET)], in_=mo[:])

            for it in range(NS * repeat):
                s = it % NS
                grp, loc = divmod(s, GT)
                if loc == 0:
                    gcols = ts(grp, GRP)
                    # ed_sb rows 0-63: X-half features; 64-127: Y-half
                    ed_sb = edgep.tile([128, GRP], EDT, name="ed_sb")
                    nd_sb = nodep.tile([128, 2 * GRP], BF16, name="nd_sb")
                    for h in range(2):
                        for r in range(2):
                            nc.sync.dma_start(
                                out=ed_sb[64 * h + 32 * r:
                                          64 * h + 32 * (r + 1), :],
                                in_=edge_ds[2 * h + r][:, gcols])
                    nc.sync.dma_start(out=nd_sb[0:32, ts(0, GRP)],
                                      in_=nodeX_d[:, gcols])
                    nc.sync.dma_start(out=nd_sb[0:32, ts(1, GRP)],
                                      in_=nodeY_d[:, gcols])
                    for c in range(1, 4):
                        # replicate across partition quadrants on-chip:
                        # SBUF->SBUF DMA costs no HBM bandwidth
                        nc.sync.dma_start(
                            out=nd_sb[32 * c:32 * (c + 1), :],
                            in_=nd_sb[0:32, :])
                lcols = ts(loc, ET)
                if not sel_loaded:
                    nc.sync.dma_start(out=sel_sb[:], in_=sel_d[:])
                    sel_loaded = True

                mgX = mpsum.tile([128, ET], F32, name="mgX", tag="mg")
                mgY = mpsum.tile([128, ET], F32, name="mgY", tag="mg")
                pps = []
                for g in range(8):
                    pi = 8 * s + g
                    mode = SCHED[pi % len(SCHED)]
                    if pi < 2 and mode != 's':
                        mode = 's'
                    ap_t = apsum.tile([128, 2 * ET], F32, name="ap_t")
                    # two concurrent 64-row PE tiles: X half on rows 0-63,
                    # Y half on rows 64-127, same W block g duplicated
                    nc.tensor.matmul(ap_t[:, ts(0, ET)],
                                     w_sb[0:64, ts(g, 128)],
                                     ed_sb[0:64, lcols],
                                     start=True, stop=True,
                                     tile_position=(0, 0))
                    nc.tensor.matmul(ap_t[:, ts(1, ET)],
                                     w_sb[64:128, ts(g, 128)],
                                     ed_sb[64:128, lcols],
                                     start=True, stop=True,
                                     tile_position=(64, 0))
                    pp = ppp.tile([128, 2 * ET], BF16, name="pp")
                    nd_b = nd_sb[:, :].rearrange(
                        "p (h e) -> p h e", h=2)[:, :, ds(loc * ET, ET)]
                    if mode == 's':
                        nc.vector.scalar_tensor_tensor(
                            out=pp[:].rearrange("p (h e) -> p h e", h=2),
                            in0=ap_t[:].rearrange("p (h e) -> p h e", h=2),
                            scalar=0.0,
                            in1=nd_b,
                            op0=mybir.AluOpType.max,
                            op1=mybir.AluOpType.mult,
                        )
                    else:
                        ar = arp.tile([128, 2 * ET], BF16, name="ar")
                        nc.scalar.activation(
                            ar[:], ap_t[:], mybir.ActivationFunctionType.Relu)
                        if mode == 'p':
                            # two half ops: halves the Pool latency so the
                            # sel run never waits on a 2us GPSIMD op
                            for hh in range(2):
                                nc.gpsimd.tensor_tensor(
                                    out=pp[:, ts(hh, ET)],
                                    in0=ar[:, ts(hh, ET)],
                                    in1=nd_b[:, hh],
                                    op=mybir.AluOpType.mult,
                                )
                        else:
                            nc.vector.tensor_tensor(
                                out=pp[:].rearrange("p (h e) -> p h e", h=2),
                                in0=ar[:].rearrange("p (h e) -> p h e", h=2),
                                in1=nd_b,
                                op=mybir.AluOpType.mult,
                            )
                    pps.append(pp)
                    if pend:
                        if g == SELX_AFTER and pend[0][4] == 0:
                            emit_sels(pend[0], 0)
                            pend[0][4] = 1
                        elif g == SELY_AFTER and pend[0][4] == 1:
                            emit_sels(pend[0], 1)
                            pend.pop(0)
                pend.append([mgX, mgY, s, pps, 0])

            while pend:
                if pend[0][4] == 0:
                    emit_sels(pend[0], 0)
                    pend[0][4] = 1
                emit_sels(pend[0], 1)
                pend.pop(0)

    nc.compile()
    return nc


def _sel_matrix() -> np.ndarray:
    """sel[p, 32*g + m] = 1 iff m == p//32 + 4*(g%2).

    Bank g holds A-columns k = 128g + p -> i = 4g + p//32.  Strip c = g//2 of
    the msg PSUM bank accumulates banks {2c, 2c+1}; its row m carries global
    i = 8c + m, and i - 8c = p//32 + 4*(g%2)."""
    sel = np.zeros((128, 8 * ND), dtype=np.float32)
    p = np.arange(128)
    for g in range(8):
        m = p // 32 + 4 * (g % 2)
        sel[p, 32 * g + m] = 1.0
    return sel.astype(ml_dtypes.bfloat16)


_LOCK = threading.Lock()
_NC = None


def _get_nc():
    global _NC
    with _LOCK:
        if _NC is None:
            _NC = _build_nc()
    return _NC


def _prep_inputs(node_j, edge_ij, W, b):
    node_j = np.asarray(node_j, dtype=np.float32)
    edge_ij = np.asarray(edge_ij, dtype=np.float32)
    W = np.asarray(W, dtype=np.float32)

    edge_flat = edge_ij.reshape(EDGES, ED)
    edgeT = np.ascontiguousarray(edge_flat.T)          # [64, EDGES]
    if EDGE_BF16:
        edgeT = edgeT.astype(ml_dtypes.bfloat16)
    nodeT = np.ascontiguousarray(
        node_j.reshape(EDGES, ND).T).astype(ml_dtypes.bfloat16)

    sel = _sel_matrix()
    w_pack = np.ascontiguousarray(np.concatenate([W, W], axis=0))  # [128, NK]
    if EDGE_BF16:
        w_pack = w_pack.astype(ml_dtypes.bfloat16)

    in_maps = []
    for c in range(N_CORES):
        lo, hi = c * E_CORE, (c + 1) * E_CORE
        mid = lo + EH
        in_maps.append({
            "edgeT00": np.ascontiguousarray(edgeT[0:32, lo:mid]),
            "edgeT01": np.ascontiguousarray(edgeT[32:64, lo:mid]),
            "edgeT10": np.ascontiguousarray(edgeT[0:32, mid:hi]),
            "edgeT11": np.ascontiguousarray(edgeT[32:64, mid:hi]),
            "nodeX": np.ascontiguousarray(nodeT[:, lo:mid]),
            "nodeY": np.ascontiguousarray(nodeT[:, mid:hi]),
            "w_pack": w_pack,
            "sel": sel,
        })
    return in_maps


def _extract_msgT(res_core: dict) -> np.ndarray:
    """[128, E_core] bf16 raw bank image -> msgT [32, E_core] fp32."""
    m = np.asarray(res_core[OUT_NAME], dtype=np.float32)
    return np.concatenate([m[32 * c:32 * c + 8] for c in range(4)], axis=0)


def kernel(node_j, edge_ij, W, b):
    b = np.asarray(b, dtype=np.float32)
    if np.any(b):
        # general-b fallback (never hit for this problem: b is zeros)
        A = np.maximum(
            edge_ij.reshape(EDGES, ED).astype(np.float32) @ np.asarray(
                W, dtype=np.float32) + b, 0.0)
        msg = np.einsum("eij,ej->ei", A.reshape(EDGES, ND, ND),
                        node_j.reshape(EDGES, ND).astype(np.float32))
        return msg.reshape(B, E_FULL, ND)
    nc = _get_nc()
    in_maps = _prep_inputs(node_j, edge_ij, W, b)
    res = run_bass_kernel_spmd(nc, in_maps, core_ids=list(range(N_CORES)))
    msgT = np.concatenate(
        [_extract_msgT(res.results[c]) for c in range(N_CORES)],
        axis=1)  # [32, EDGES]
    return np.ascontiguousarray(msgT.T).reshape(B, E_FULL, ND)


# revision 32
# speedup vs baseline: 1.0622x; 1.0622x over previous
"""Trainium2 Bass kernel for nn_Message_Passer (gnn_message_passing).

Reference computation:
    A = relu(edge_ij @ W + b)            # [B, E, 1024]
    messages = einsum("beij,bej->bei", A.reshape(B,E,32,32), node_j)

Row-tiled design (8 NeuronCores, data-parallel over the B*E edge dim):
  - matmul1 uses PE row tiling: contraction is the 64 edge features (the bias
    is dropped — b is zeros for this problem; a numpy fallback handles b != 0)
    and the 128x128 PE array runs two concurrent 64-row tiles: rows 0-63
    process the X half of the core's edges (0..E/2), rows 64-127 the Y half,
    with W duplicated across both row halves.  Each PSUM pair is [128, 2*ET]
    = A-bank g for (X-tile | Y-tile); a super-tile is 8 pairs covering all 8
    W-blocks for 1024 edges.
  - Fused relu+multiply P = max(A,0) * node_rep, split across engines by a
    per-pair schedule: DVE scalar_tensor_tensor straight from PSUM ('s'),
    ACT relu + DVE bf16 multiply ('a'), ACT relu + GPSIMD multiply ('p') —
    balanced so DVE/ACT/Pool all stay busy.
  - j-reduction on the PE: 0/1 selector matmuls, col-tiled 4-wide and emitted
    as strip-interleaved runs one super-tile behind the fills so the in-order
    PE queue never waits on the elementwise engines.
  - Per half-tile: one [128, ET] PSUM->SBUF bf16 copy + one full-bank DMA
    store (partial-partition strip DMAs silently corrupt — DMA-queue race).
"""

import threading

import numpy as np
import ml_dtypes

import concourse.bass as bass
import concourse.mybir as mybir
import concourse.tile as tile
from concourse import bacc
from concourse.bass import ts, ds
from concourse.bass_utils import run_bass_kernel_spmd

N_CORES = 8
B, E_FULL, ND, ED = 16, 4096, 32, 64
EDGES = B * E_FULL            # 65536
E_CORE = EDGES // N_CORES     # 8192
EH = E_CORE // 2              # 4096 edges per half (X / Y)
ET = 512                      # edges per on-chip tile (per half)
NS = EH // ET                 # 8 super-tiles
GT = 4                        # super-tiles per input-load group
GRP = GT * ET                 # 2048 cols per half per load group
KDIM = ED                     # 64 (features; contraction dim)
NK = ND * ND                  # 1024 A-columns
F32 = mybir.dt.float32
F32R = mybir.dt.float32r
BF16 = mybir.dt.bfloat16

OUT_NAME = "msg_out"

# Per bank-pair engine schedule (cycled): 's' = DVE scalar_tensor_tensor,
# 'a' = ACT relu + DVE mult, 'p' = ACT relu + GPSIMD mult.  Tuned on HW.
SCHED = list("saasap")
# Engine for each msg PSUM->SBUF copy (cycled): 'A' = ACT, 'V' = DVE.
COPY_SCHED = list("AAV")
# After which pair of super s+1 the X-sel run (resp. Y) of super s is emitted
SELX_AFTER = 0
SELY_AFTER = 1
# fp32r edges/weights measured faster than bf16 (bf16 FWL weight loads
# contend for XBUSes with the col-tiled selector matmuls)
EDGE_BF16 = False
EDT = BF16 if EDGE_BF16 else F32R


def _build_nc(repeat: int = 1):
    nc = bacc.Bacc("TRN2", target_bir_lowering=False, debug=False,
                   num_devices=N_CORES)
    # edge features split into 32-row tensors per half (proven-safe DMA
    # shapes: full-row-range source, col offsets <= 24KB)
    edge_ds = [nc.dram_tensor(f"edgeT{h}{r}", [32, EH], EDT,
                              kind="ExternalInput")
               for h in range(2) for r in range(2)]  # X0,X1,Y0,Y1
    nodeX_d = nc.dram_tensor("nodeX", [ND, EH], BF16, kind="ExternalInput")
    nodeY_d = nc.dram_tensor("nodeY", [ND, EH], BF16, kind="ExternalInput")
    w_d = nc.dram_tensor("w_pack", [128, NK], EDT, kind="ExternalInput")
    sel_d = nc.dram_tensor("sel", [128, 8 * ND], BF16, kind="ExternalInput")
    out_d = nc.dram_tensor(OUT_NAME, [128, E_CORE], BF16, kind="ExternalOutput")

    with tile.TileContext(nc) as tc:
        with (
            tc.tile_pool(name="const", bufs=1) as constp,
            tc.tile_pool(name="edge", bufs=3) as edgep,
            tc.tile_pool(name="node", bufs=3) as nodep,
            tc.tile_pool(name="ar", bufs=6) as arp,
            tc.tile_pool(name="pp", bufs=12) as ppp,
            tc.tile_pool(name="mo", bufs=6) as mop,
            tc.tile_pool(name="apsum", bufs=3, space="PSUM") as apsum,
            tc.tile_pool(name="mpsum", bufs=2, space="PSUM") as mpsum,
        ):
            w_sb = constp.tile([128, NK], EDT, name="w_sb")
            nc.sync.dma_start(out=w_sb[:], in_=w_d[:])
            sel_sb = constp.tile([128, 8 * ND], BF16, name="sel_sb")
            sel_loaded = False

            # pending super-tile sel work: [mgX, mgY, s, pps, stage]
            pend = []

            def emit_sels(entry, half):
                mgs, s_, pps = entry[0:2], entry[2], entry[3]
                mg_ = mgs[half]
                # even banks (start) then odd banks (stop): runs of 4
                # distinct col-tile positions overlap on the PE array
                for par in range(2):
                    for c_ in range(4):
                        g_ = 2 * c_ + par
                        nc.tensor.matmul(mg_[32 * c_:32 * (c_ + 1), :],
                                         sel_sb[:, ts(g_, ND)],
                                         pps[g_][:, ts(half, ET)],
                                         start=(par == 0), stop=(par == 1),
                                         skip_group_check=True,
                                         tile_position=(0, 32 * c_))
                # copy + store this half's messages
                mo = mop.tile([128, ET], BF16, name="mo")
                if COPY_SCHED[(2 * s_ + half) % len(COPY_SCHED)] == 'V':
                    nc.vector.tensor_copy(mo[:], mg_[:])
                else:
                    nc.scalar.copy(mo[:], mg_[:])
                nc.sync.dma_start(
                    out=out_d[:, ts(half * NS + s_, ET)], in_=mo[:])

            for it in range(NS * repeat):
                s = it % NS
                grp, loc = divmod(s, GT)
                if loc == 0:
                    gcols = ts(grp, GRP)
                    # ed_sb rows 0-63: X-half features; 64-127: Y-half
                    ed_sb = edgep.tile([128, GRP], EDT, name="ed_sb")
                    nd_sb = nodep.tile([128, 2 * GRP], BF16, name="nd_sb")
                    for h in range(2):
                        for r in range(2):
                            nc.sync.dma_start(
                                out=ed_sb[64 * h + 32 * r:
                                          64 * h + 32 * (r + 1), :],
                                in_=edge_ds[2 * h + r][:, gcols])
                    nc.sync.dma_start(out=nd_sb[0:32, ts(0, GRP)],
                                      in_=nodeX_d[:, gcols])
                    nc.sync.dma_start(out=nd_sb[0:32, ts(1, GRP)],
                                      in_=nodeY_d[:, gcols])
                    for c in range(1, 4):
                        # replicate across partition quadrants on-chip:
                        # SBUF->SBUF DMA costs no HBM bandwidth
                        nc.sync.dma_start(
                            out=nd_sb[32 * c:32 * (c + 1), :],
                            in_=nd_sb[0:32, :])
                lcols = ts(loc, ET)
                if not sel_loaded:
                    nc.sync.dma_start(out=sel_sb[:], in_=sel_d[:])
                    sel_loaded = True

                mgX = mpsum.tile([128, ET], F32, name="mgX", tag="mg")
                mgY = mpsum.tile([128, ET], F32, name="mgY", tag="mg")
                pps = []
                for g in range(8):
                    pi = 8 * s + g
                    mode = SCHED[pi % len(SCHED)]
                    if pi < 2 and mode != 's':
                        mode = 's'
                    ap_t = apsum.tile([128, 2 * ET], F32, name="ap_t")
                    # two concurrent 64-row PE tiles: X half on rows 0-63,
                    # Y half on rows 64-127, same W block g duplicated
                    nc.tensor.matmul(ap_t[:, ts(0, ET)],
                                     w_sb[0:64, ts(g, 128)],
                                     ed_sb[0:64, lcols],
                                     start=True, stop=True,
                                     tile_position=(0, 0))
                    nc.tensor.matmul(ap_t[:, ts(1, ET)],
                                     w_sb[64:128, ts(g, 128)],
                                     ed_sb[64:128, lcols],
                                     start=True, stop=True,
                                     tile_position=(64, 0))
                    pp = ppp.tile([128, 2 * ET], BF16, name="pp")
                    nd_b = nd_sb[:, :].rearrange(
                        "p (h e) -> p h e", h=2)[:, :, ds(loc * ET, ET)]
                    if mode == 's':
                        nc.vector.scalar_tensor_tensor(
                            out=pp[:].rearrange("p (h e) -> p h e", h=2),
                            in0=ap_t[:].rearrange("p (h e) -> p h e", h=2),
                            scalar=0.0,
                            in1=nd_b,
                            op0=mybir.AluOpType.max,
                            op1=mybir.AluOpType.mult,
                        )
                    else:
                        ar = arp.tile([128, 2 * ET], BF16, name="ar")
                        nc.scalar.activation(
                            ar[:], ap_t[:], mybir.ActivationFunctionType.Relu)
                        eng = nc.gpsimd if mode == 'p' else nc.vector
                        eng.tensor_tensor(
                            out=pp[:].rearrange("p (h e) -> p h e", h=2),
                            in0=ar[:].rearrange("p (h e) -> p h e", h=2),
                            in1=nd_b,
                            op=mybir.AluOpType.mult,
                        )
                    pps.append(pp)
                    if pend:
                        if g == SELX_AFTER and pend[0][4] == 0:
                            emit_sels(pend[0], 0)
                            pend[0][4] = 1
                        elif g == SELY_AFTER and pend[0][4] == 1:
                            emit_sels(pend[0], 1)
                            pend.pop(0)
                pend.append([mgX, mgY, s, pps, 0])

            while pend:
                if pend[0][4] == 0:
                    emit_sels(pend[0], 0)
                    pend[0][4] = 1
                emit_sels(pend[0], 1)
                pend.pop(0)

    nc.compile()
    return nc


def _sel_matrix() -> np.ndarray:
    """sel[p, 32*g + m] = 1 iff m == p//32 + 4*(g%2).

    Bank g holds A-columns k = 128g + p -> i = 4g + p//32.  Strip c = g//2 of
    the msg PSUM bank accumulates banks {2c, 2c+1}; its row m carries global
    i = 8c + m, and i - 8c = p//32 + 4*(g%2)."""
    sel = np.zeros((128, 8 * ND), dtype=np.float32)
    p = np.arange(128)
    for g in range(8):
        m = p // 32 + 4 * (g % 2)
        sel[p, 32 * g + m] = 1.0
    return sel.astype(ml_dtypes.bfloat16)


_LOCK = threading.Lock()
_NC = None


def _get_nc():
    global _NC
    with _LOCK:
        if _NC is None:
            _NC = _build_nc()
    return _NC


def _prep_inputs(node_j, edge_ij, W, b):
    node_j = np.asarray(node_j, dtype=np.float32)
    edge_ij = np.asarray(edge_ij, dtype=np.float32)
    W = np.asarray(W, dtype=np.float32)

    edge_flat = edge_ij.reshape(EDGES, ED)
    edgeT = np.ascontiguousarray(edge_flat.T)          # [64, EDGES]
    if EDGE_BF16:
        edgeT = edgeT.astype(ml_dtypes.bfloat16)
    nodeT = np.ascontiguousarray(
        node_j.reshape(EDGES, ND).T).astype(ml_dtypes.bfloat16)

    sel = _sel_matrix()
    w_pack = np.ascontiguousarray(np.concatenate([W, W], axis=0))  # [128, NK]
    if EDGE_BF16:
        w_pack = w_pack.astype(ml_dtypes.bfloat16)

    in_maps = []
    for c in range(N_CORES):
        lo, hi = c * E_CORE, (c + 1) * E_CORE
        mid = lo + EH
        in_maps.append({
            "edgeT00": np.ascontiguousarray(edgeT[0:32, lo:mid]),
            "edgeT01": np.ascontiguousarray(edgeT[32:64, lo:mid]),
            "edgeT10": np.ascontiguousarray(edgeT[0:32, mid:hi]),
            "edgeT11": np.ascontiguousarray(edgeT[32:64, mid:hi]),
            "nodeX": np.ascontiguousarray(nodeT[:, lo:mid]),
            "nodeY": np.ascontiguousarray(nodeT[:, mid:hi]),
            "w_pack": w_pack,
            "sel": sel,
        })
    return in_maps


def _extract_msgT(res_core: dict) -> np.ndarray:
    """[128, E_core] bf16 raw bank image -> msgT [32, E_core] fp32."""
    m = np.asarray(res_core[OUT_NAME], dtype=np.float32)
    return np.concatenate([m[32 * c:32 * c + 8] for c in range(4)], axis=0)


def kernel(node_j, edge_ij, W, b):
    b = np.asarray(b, dtype=np.float32)
    if np.any(b):
        # general-b fallback (never hit for this problem: b is zeros)
        A = np.maximum(
            edge_ij.reshape(EDGES, ED).astype(np.float32) @ np.asarray(
                W, dtype=np.float32) + b, 0.0)
        msg = np.einsum("eij,ej->ei", A.reshape(EDGES, ND, ND),
                        node_j.reshape(EDGES, ND).astype(np.float32))
        return msg.reshape(B, E_FULL, ND)
    nc = _get_nc()
    in_maps = _prep_inputs(node_j, edge_ij, W, b)
    res = run_bass_kernel_spmd(nc, in_maps, core_ids=list(range(N_CORES)))
    msgT = np.concatenate(
        [_extract_msgT(res.results[c]) for c in range(N_CORES)],
        axis=1)  # [32, EDGES]
    return np.ascontiguousarray(msgT.T).reshape(B, E_FULL, ND)
